# revision 36
# baseline (speedup 1.0000x reference)
"""Bass/Tile kernel for nn_Attention_81690277970645 on TRN2 (v5).

Sharding: 8 cores = 2 batches x 4 head-groups (4 heads of d=64 each).
Per core (batch bi, head-group hg):
  inputs:  x_b [2048, 1024], wq/wk/wv slices [1024, 256], bq [256],
           wo slice [256, 1024]
  output:  partial out [2048, 1024] (host sums the 4 head-group partials
           per batch and adds bo + bv @ wo)

The kernel is ScalarE(exp)-bound: 128 exp ops of [128,1024] at ~1.05us
each.  Everything is organized around a continuous exp stream:
  - one global stream of 128 (hp, ib, jt) slots; S/exp leads the
    AV+filler stream by 2 slots (psS double-buffer bound)
  - pass 1 (hp0, ib0) streams against the x DMA: per 4-jt chunk, cast
    (DVE) + xbar-transpose straight into xT, then project that KT chunk
  - all other QKV projections / output projections are PE fillers split
    into half-units so no burst exceeds ~1.2us
  - bq applied via a K=1 ones-row matmul into the QT psum accumulation
  - bf16 everywhere; bk dropped (softmax-invariant); bv folded into the
    host-side gather; row-packed S pairs (K=64 x2 concurrent)
  - divide tail: den broadcast via PE, reciprocal_approx_fast, then
    tensor_tensor reading the AV psum directly
"""
import sys
import numpy as np

if '/opt/trn_rl_repo' not in sys.path:
    sys.path.insert(0, '/opt/trn_rl_repo')

import concourse.mybir as mybir
from concourse import bacc
from concourse.tile import TileContext

F32 = mybir.dt.float32
F32R = mybir.dt.float32r
BF16 = mybir.dt.bfloat16

SEQ = 2048
DIM = 1024
EMB_C = 256          # per-core emb columns (4 heads x 64)
NH = 4               # heads per core
DH = 64
SCALE = DH ** -0.5
P = 128
NSEQT = SEQ // P     # 16 seq tiles
NDIMC = DIM // P     # 8 dim chunks
NEMBC = EMB_C // P   # 2 emb chunks (= head pairs)
IBLK = 512
NIBLK = SEQ // IBLK  # 4 i-blocks
NJT = SEQ // P       # 16 j tiles
ID = mybir.ActivationFunctionType.Identity
EXP = mybir.ActivationFunctionType.Exp


def build_kernel(row_pack=True):
    nc = bacc.Bacc("TRN2", target_bir_lowering=False, debug=False, num_devices=8)

    x = nc.dram_tensor("x", [SEQ, DIM], F32, kind="ExternalInput")
    wq = nc.dram_tensor("wq", [DIM, EMB_C], F32, kind="ExternalInput")
    wk = nc.dram_tensor("wk", [DIM, EMB_C], F32, kind="ExternalInput")
    wv = nc.dram_tensor("wv", [DIM, EMB_C], F32, kind="ExternalInput")
    bq = nc.dram_tensor("bq", [EMB_C], F32, kind="ExternalInput")
    wo = nc.dram_tensor("wo", [EMB_C, DIM], F32, kind="ExternalInput")
    ones_d = nc.dram_tensor("ones64", [P, DH], F32, kind="ExternalInput")
    out = nc.dram_tensor("out", [SEQ, DIM], F32, kind="ExternalOutput")

    with TileContext(nc) as tc:
        with (
            tc.tile_pool(name="const", bufs=1) as const_pool,
            tc.tile_pool(name="w", bufs=1) as w_pool,
            tc.tile_pool(name="big", bufs=1) as big_pool,
            tc.tile_pool(name="stage", bufs=3) as stage_pool,
            tc.tile_pool(name="ps", bufs=1, space="PSUM") as ps,
        ):
            # ---- x DMAs (per tile, for parallel queue bandwidth) ----
            xs_tiles = [None] * NSEQT

            def emit_xdma(s):
                xs = stage_pool.tile([P, DIM], F32, tag="xs", bufs=14,
                                     name=f"xs_{s}")
                nc.sync.dma_start(xs[:], x[s * P:(s + 1) * P, :])
                xs_tiles[s] = xs

            for s in range(NSEQT):
                emit_xdma(s)
            ones_t = const_pool.tile([P, DH], F32R)
            nc.sync.dma_start(ones_t[:], ones_d[:].bitcast(F32R))

            # weights via the (idle) ScalarE HWDGE queue, split in halves
            # for parallel queues; casts on DVE
            w_sbs = {}

            def emit_wdma(name, wd, cast_eng='act', defer_cast=False):
                wf = stage_pool.tile([P, NDIMC, EMB_C], F32, tag="wstage",
                                     bufs=3, name=f"wf_{name}")
                src = wd.rearrange("(c p) e -> p c e", p=P)
                nc.scalar.dma_start(wf[:, :4, :], src[:, :4, :])
                nc.scalar.dma_start(wf[:, 4:, :], src[:, 4:, :])
                wb = w_pool.tile([P, NDIMC, EMB_C], BF16, name=f"wb_{name}")
                w_sbs[name] = wb

                def cast():
                    if cast_eng == 'act':
                        nc.scalar.copy(wb[:], wf[:])
                    else:
                        nc.vector.tensor_copy(wb[:], wf[:])
                if defer_cast:
                    return cast
                cast()

            wv_cast = emit_wdma("wv", wv, cast_eng='vector', defer_cast=True)
            emit_wdma("wk", wk)
            emit_wdma("wq", wq)
            bq_f = w_pool.tile([1, EMB_C], F32, name="bq_f")
            nc.sync.dma_start(bq_f[:], bq.rearrange("(a e) -> a e", a=1))
            bq_bf = w_pool.tile([1, EMB_C], BF16, name="bq_bf")
            nc.scalar.copy(bq_bf[:], bq_f[:])
            ones_row = const_pool.tile([1, IBLK], BF16)
            nc.vector.memset(ones_row[:], 1.0)

            # ---- big persistent tiles ----
            xT = big_pool.tile([P, NDIMC, SEQ], BF16)    # [dim_low, chunk, seq]
            QT = big_pool.tile([P, NEMBC, SEQ], BF16)
            KT = big_pool.tile([P, NEMBC, SEQ], BF16)
            VP = big_pool.tile([P, NSEQT, NH * (DH + 1)], BF16)
            outT = big_pool.tile([P, NEMBC, SEQ], BF16)
            for h in range(NH):
                nc.vector.memset(VP[:, :, h * (DH + 1) + DH], 1.0)

            # ---- helpers ----
            def emit_xform(k):
                """cast + transpose x tiles 4k..4k+3 into xT."""
                for s in range(4 * k, 4 * k + 4):
                    xb = stage_pool.tile([P, DIM], BF16, tag="xb", bufs=5,
                                         name=f"xb_{s}")
                    nc.vector.tensor_copy(xb[:], xs_tiles[s][:])
                    nc.sync.dma_start_transpose(
                        xT[:, :, s * P:(s + 1) * P], xb[:])

            def emit_proj(dst, wname, pair, col0, ncols, eng):
                """dst[:, pair, col0:+ncols] = w^T @ xT (+bq via ones-row MM)."""
                wb = w_sbs[wname]
                is_q = dst is QT
                pq = ps.tile([P, IBLK], F32, tag="po", bufs=2,
                             name=f"pq_{wname}_{pair}_{col0}")
                for c in range(NDIMC):
                    nc.tensor.matmul(
                        pq[:, :ncols],
                        wb[:, c, pair * P:(pair + 1) * P],
                        xT[:, c, col0:col0 + ncols],
                        start=(c == 0),
                        stop=(c == NDIMC - 1 and not is_q),
                    )
                if is_q:
                    nc.tensor.matmul(
                        pq[:, :ncols],
                        bq_bf[0:1, pair * P:(pair + 1) * P],
                        ones_row[0:1, :ncols],
                        start=False, stop=True,
                    )
                d = dst[:, pair, col0:col0 + ncols]
                if eng == 'act':
                    nc.scalar.activation(d, pq[:, :ncols], ID, bias=0.0,
                                         scale=1.0)
                else:
                    nc.vector.tensor_copy(d, pq[:, :ncols])

            def proj_parts(dst, wname, pair, col0, ncols):
                h = ncols // 2
                return [
                    lambda: emit_proj(dst, wname, pair, col0, h, 'vector'),
                    lambda: emit_proj(dst, wname, pair, col0 + h, h, 'vector'),
                ]

            def emit_vp(s):
                pv = ps.tile([P, EMB_C], F32, tag="po", bufs=2, name=f"pv_{s}")
                for c in range(NDIMC):
                    nc.tensor.matmul(
                        pv[:],
                        xT[:, c, s * P:(s + 1) * P],
                        w_sbs["wv"][:, c, :],
                        start=(c == 0), stop=(c == NDIMC - 1),
                    )
                nc.vector.tensor_copy(
                    VP[:, s, :].rearrange("p (h x) -> p h x", h=NH)[:, :, :DH],
                    pv[:].rearrange("p (h d) -> p h d", h=NH),
                )

            def emit_spair(ib, jt, hp):
                i0 = ib * IBLK
                psS = ps.tile([P, 2, IBLK], F32, tag="s0", bufs=2,
                              name=f"ps{hp}_{ib}_{jt}")
                for hh in range(2):
                    lo = hh * DH
                    nc.tensor.matmul(
                        psS[:, hh, :],
                        KT[lo:lo + DH, hp, jt * P:(jt + 1) * P],
                        QT[lo:lo + DH, hp, i0:i0 + IBLK],
                        start=True, stop=True,
                    )
                es = stage_pool.tile([P, 2, IBLK], BF16, tag="es", bufs=5,
                                     name=f"es{hp}_{ib}_{jt}")
                nc.scalar.activation(es[:], psS[:], EXP, bias=0.0, scale=SCALE)
                return es

            def emit_av(pavs, es, jt, hp):
                for hh in range(2):
                    h = hp * 2 + hh
                    nc.tensor.matmul(
                        pavs[hh][:DH + 1, :],
                        VP[:, jt, h * (DH + 1):(h + 1) * (DH + 1)],
                        es[:, hh, :],
                        start=(jt == 0), stop=(jt == NJT - 1),
                    )

            def make_div(h, ib, pav, den_row):
                i0 = ib * IBLK

                def go():
                    recb_ps = ps.tile([P, IBLK], F32, tag="po", bufs=2,
                                      name=f"recb_{h}_{ib}")
                    nc.tensor.matmul(
                        recb_ps[:DH, :], ones_t[0:1, :], den_row[:],
                        start=True, stop=True,
                    )
                    recb_sb = stage_pool.tile([DH, IBLK], F32, tag="recb",
                                              bufs=2)
                    nc.vector.reciprocal_approx_fast(recb_sb[:], recb_ps[:DH, :])
                    e_c, e_lo = divmod(h * DH, P)
                    nc.vector.tensor_tensor(
                        outT[e_lo:e_lo + DH, e_c, i0:i0 + IBLK],
                        pav[:DH, :], recb_sb[:], mybir.AluOpType.mult,
                    )
                return go

            def finalize_pass(hp, ib, pavs):
                for hh in range(2):
                    h = hp * 2 + hh
                    den_row = stage_pool.tile([1, IBLK], F32R, tag="den_row",
                                              bufs=2, name=f"den_{h}_{ib}")
                    nc.vector.tensor_copy(
                        den_row[:], pavs[hh][DH:DH + 1, :].bitcast(F32R))
                    div2.append(make_div(h, ib, pavs[hh], den_row))

            def oproj_units(ib, tail=False):
                units = []
                for s in range(ib * (IBLK // P), (ib + 1) * (IBLK // P)):
                    def go(s=s, tail=tail):
                        oc = stage_pool.tile([P, DIM], F32, tag="oc", bufs=2,
                                             name=f"oc_{s}")
                        for nb in range(DIM // IBLK):
                            po = ps.tile([P, IBLK], F32, tag="po", bufs=2,
                                         name=f"po_{s}_{nb}")
                            for e in range(NEMBC):
                                nc.tensor.matmul(
                                    po[:],
                                    outT[:, e, s * P:(s + 1) * P],
                                    wo_sb[:, e, nb * IBLK:(nb + 1) * IBLK],
                                    start=(e == 0), stop=(e == NEMBC - 1),
                                )
                            d = oc[:, nb * IBLK:(nb + 1) * IBLK]
                            if tail and nb == 0:
                                nc.scalar.copy(d, po[:])
                            else:
                                nc.vector.tensor_copy(d, po[:])
                        nc.sync.dma_start(out[s * P:(s + 1) * P, :], oc[:])
                    units.append(go)
                return units

            # ---- prologue: chunk 0 + first projections ----
            emit_xform(0)
            wv_cast()
            emit_proj(KT, "wk", 0, 0, 256, 'act')
            emit_proj(KT, "wk", 0, 256, 256, 'act')
            emit_proj(QT, "wq", 0, 0, IBLK, 'act')
            wof = stage_pool.tile([P, NEMBC, DIM], F32, tag="wostage", bufs=1,
                                  name="wf_wo")
            wo_sb = w_pool.tile([P, NEMBC, DIM], BF16, name="wb_wo")

            def wo_load():
                src = wo.rearrange("(c p) n -> p c n", p=P)
                nc.sync.dma_start(wof[:, 0, :], src[:, 0, :])
                nc.sync.dma_start(wof[:, 1, :], src[:, 1, :])

            # fillers (split half-units), popped one per even jt
            fill = [wo_load,
                    lambda: nc.vector.tensor_copy(wo_sb[:], wof[:])]
            for k in range(4):
                fill.extend(proj_parts(KT, "wk", 1, k * IBLK, IBLK))
            for k in range(4):
                fill.extend(proj_parts(QT, "wq", 1, k * IBLK, IBLK))

            div2 = []
            pending = []
            slots = [(hp, ib, jt)
                     for hp in range(2)
                     for ib in range(NIBLK)
                     for jt in range(NJT)]
            N = len(slots)
            es_of = [None] * N
            pavs_of = {}

            def s_emit(t):
                hp, ib, jt = slots[t]
                if hp == 0 and ib == 0 and jt in (4, 8, 12):
                    k = jt // 4
                    emit_xform(k)
                    emit_proj(KT, "wk", 0, k * IBLK, IBLK, 'vector')
                es_of[t] = emit_spair(ib, jt, hp)

            def av_emit(t):
                hp, ib, jt = slots[t]
                if jt == 0:
                    pavs_of[(hp, ib)] = [
                        ps.tile([P, IBLK], F32, tag="pav", bufs=2,
                                name=f"pav_{hp}_{hh}_{ib}")
                        for hh in range(2)
                    ]
                emit_av(pavs_of[(hp, ib)], es_of[t], jt, hp)
                es_of[t] = None
                if jt == NJT - 1:
                    finalize_pass(hp, ib, pavs_of.pop((hp, ib)))
                    if hp == 1:
                        pending.extend(oproj_units(ib))

            def extras(t):
                hp, ib, jt = slots[t]
                if div2 and jt < 2:
                    div2.pop(0)()
                first = (hp == 0 and ib == 0)
                if first:
                    emit_vp(jt)
                    if jt in (10, 11):
                        emit_proj(QT, "wq", 0, IBLK + (jt - 10) * 256, 256,
                                  'vector')
                else:
                    if hp == 0 and jt in (10, 11) and ib < NIBLK - 1:
                        emit_proj(QT, "wq", 0,
                                  (ib + 1) * IBLK + (jt - 10) * 256, 256,
                                  'vector')
                    if fill and jt % 2 == 0 and jt >= 2:
                        fill.pop(0)()
                    if pending and (jt % 2 == 1 or
                                    (hp == 1 and ib == NIBLK - 1 and jt >= 2)):
                        pending.pop(0)()

            # S/exp leads the AV+filler stream by 2 slots
            s_emit(0)
            s_emit(1)
            for t in range(N):
                if t + 2 < N:
                    s_emit(t + 2)
                if t >= 1:
                    av_emit(t - 1)
                extras(t)

            # drain
            av_emit(N - 1)
            for go in div2:
                go()
            for go in pending:
                go()
            for go in oproj_units(NIBLK - 1, tail=True):
                go()

    nc.compile()
    return nc


def shard_inputs(inputs):
    """Full inputs dict -> list of 8 per-core input dicts."""
    x = np.ascontiguousarray(inputs["x"], dtype=np.float32)
    maps = []
    for core in range(8):
        bi, hg = divmod(core, 4)
        sl = slice(hg * EMB_C, (hg + 1) * EMB_C)
        maps.append({
            "x": np.ascontiguousarray(x[bi]),
            "wq": np.ascontiguousarray(inputs["wq"][:, sl], np.float32),
            "wk": np.ascontiguousarray(inputs["wk"][:, sl], np.float32),
            "wv": np.ascontiguousarray(inputs["wv"][:, sl], np.float32),
            "bq": np.ascontiguousarray(inputs["bq"][sl], np.float32),
            "wo": np.ascontiguousarray(inputs["wo"][sl, :], np.float32),
            "ones64": np.ones((P, DH), np.float32),
        })
    return maps


def gather_outputs(results, inputs):
    out = np.zeros((2, SEQ, DIM), np.float32)
    for core in range(8):
        bi = core // 4
        out[bi] += results[core]["out"]
    bo = np.asarray(inputs["bo"], np.float32)
    bv = np.asarray(inputs["bv"], np.float32)
    wo = np.asarray(inputs["wo"], np.float32)
    out += bo + bv @ wo
    return out


_NC_CACHE = {}


def _get_nc(row_pack=True):
    if row_pack not in _NC_CACHE:
        _NC_CACHE[row_pack] = build_kernel(row_pack=row_pack)
    return _NC_CACHE[row_pack]


def run_sharded(inputs, trace=False, row_pack=True):
    """Returns (full_output [2,2048,1024] fp32, BassKernelResults)."""
    from concourse import bass_utils
    nc = _get_nc(row_pack)
    maps = shard_inputs(inputs)
    res = bass_utils.run_bass_kernel_spmd(
        nc, maps, core_ids=list(range(8)), trace=trace,
    )
    out = gather_outputs(res.results, inputs)
    return out, res


def kernel(**inputs):
    out, _ = run_sharded(inputs)
    return out
